# revision 18
# baseline (speedup 1.0000x reference)
"""nn_Attention kernel — dense transformer attention block on 8 TRN2 NeuronCores.

Contract: kernel(**inputs) takes FULL unsharded inputs and returns the FULL
output. Sharding: tensor-parallel over heads x data-parallel over batch.
Core c = bi*4 + hg handles batch bi, heads 4*hg..4*hg+3. Each core computes
its QKV column slice, RoPE, causal attention, and a row-slice partial of the
output projection; partials are summed on the host (the all-reduce) and
batches concatenated.

Kernel layout notes:
 - q,k are produced directly in [d, t] layout (W stationary, xT moving) so
   the S^T = K Q^T matmul needs no transposes. v is produced in [t, d]
   (xT stationary) to serve as the PV lhsT.
 - RoPE: host permutes W_q/W_k columns per head to [evens, odds, pass] and
   the first QKV M-tile is [q1 | k1 | q2 | k2] (4 heads x 8 rows each), so
   rotation is 4 batched DVE muls + 16 small writes into the final tiles.
   The d-permutation cancels in q.k dot products.
 - Softmax is computed unnormalized (exp, no max subtraction — scores are
   O(1) so exp cannot overflow); a ones column appended to V makes the
   denominator row 64 of the PV output, and normalization is applied after
   PV via reciprocal + partition_broadcast.
 - Causality: tk tiles strictly above the tq block are skipped outright;
   the 4 diagonal tiles per block are fixed with a binary mask multiply.
"""

import os
import sys
import types

import numpy as np

try:
    import ml_dtypes

    _BF16 = ml_dtypes.bfloat16
except Exception:  # pragma: no cover
    _BF16 = None

D_HEAD = 64
N_HEADS = 16
ROT_DIM = 16  # D_HEAD * 0.25
B, T, C = 2, 2048, N_HEADS * D_HEAD
HPC = 4  # heads per core
N_CORES = 8
TB = 512  # tq block size
NTB = T // TB  # 4 tq blocks
NKT = T // 128  # 16 tk tiles

_nc_cache = {}
LAST_EXEC_NS = None


# ----------------------------------------------------------------------------
# numpy reference fallback (used only if the mask is not causal-tril)
# ----------------------------------------------------------------------------

def _bf(a):
    if _BF16 is None:
        return np.float32(a)
    return np.asarray(a).astype(_BF16).astype(np.float32)


def _softmax_f32(s):
    s = s - s.max(axis=-1, keepdims=True)
    e = np.exp(s, dtype=np.float32)
    return e / e.sum(axis=-1, keepdims=True)


def _kernel_numpy(x, mask, w_qkv, w_out, rope_sin, rope_cos):
    x = np.asarray(x, dtype=np.float32)
    mask = np.asarray(mask)
    sin = _bf(np.asarray(rope_sin, dtype=np.float32))
    cos = _bf(np.asarray(rope_cos, dtype=np.float32))
    b, t, c = x.shape
    add_mask = (1.0 - mask.astype(np.float32)) * -1e10
    xb = _bf(x)
    wq = _bf(np.asarray(w_qkv, dtype=np.float32))
    wo = _bf(np.asarray(w_out, dtype=np.float32))
    qkv = _bf(np.einsum("btc,cd->btd", xb, wq, optimize=True))
    qkv = qkv.reshape(b, t, 3, N_HEADS, D_HEAD)
    q, k, v = qkv[:, :, 0], qkv[:, :, 1], qkv[:, :, 2]
    sin4 = sin[None, :, None, :]
    cos4 = cos[None, :, None, :]

    def rope(z):
        z_rot, z_pass = z[..., :ROT_DIM], z[..., ROT_DIM:]
        z1, z2 = z_rot[..., 0::2], z_rot[..., 1::2]
        zr = np.concatenate(
            [_bf(_bf(z1 * cos4) - _bf(z2 * sin4)),
             _bf(_bf(z1 * sin4) + _bf(z2 * cos4))], axis=-1
        )
        return np.concatenate([zr, z_pass], axis=-1)

    q = rope(q)
    k = rope(k)
    out_heads = np.empty((b, t, N_HEADS, D_HEAD), dtype=np.float32)
    scale = 1.0 / np.sqrt(np.float32(D_HEAD))
    for bi in range(b):
        for h in range(N_HEADS):
            s = _bf(q[bi, :, h] @ k[bi, :, h].T)
            s = _bf(s * scale)
            s = s.astype(np.float32) + add_mask[0, 0]
            p = _bf(_softmax_f32(s))
            out_heads[bi, :, h] = _bf(p @ v[bi, :, h])
    out = out_heads.reshape(b * t, c)
    y = _bf(out @ wo)
    return y.reshape(b, t, c).astype(np.float32)


# ----------------------------------------------------------------------------
# bass kernel
# ----------------------------------------------------------------------------

def _install_ntff_hook():
    """Shim antenv.axon_hooks (missing in this image) so trace=True works."""
    if "antenv.axon_hooks" in sys.modules:
        return
    hook = [None]
    mod = types.ModuleType("antenv.axon_hooks")
    mod.set_axon_ntff_profile_hook = lambda h: hook.__setitem__(0, h)
    mod.get_axon_ntff_profile_hook = lambda: hook[0]
    sys.modules["antenv.axon_hooks"] = mod
    try:
        from trn_agent_boot.trn_boot import _ntff_profile_via_ctypes

        mod.set_axon_ntff_profile_hook(
            _ntff_profile_via_ctypes("/opt/axon/libaxon_pjrt.so")
        )
    except Exception:
        pass


def _build_bass():
    """Build + compile the single-core SPMD program (same for all 8 cores)."""
    if "nc" in _nc_cache:
        return _nc_cache["nc"]

    import concourse.bass as bass
    import concourse.mybir as mybir
    import concourse.tile as tile
    from concourse import bacc

    fp32 = mybir.dt.float32
    bf16 = mybir.dt.bfloat16

    nc = bacc.Bacc("TRN2", target_bir_lowering=False, debug=False,
                   num_devices=N_CORES)

    xT_d = nc.dram_tensor("xT", [C, T], bf16, kind="ExternalInput")
    wqkv_d = nc.dram_tensor("wqkv", [C, 896], bf16, kind="ExternalInput")
    wout_d = nc.dram_tensor("wout", [HPC * D_HEAD, C], bf16, kind="ExternalInput")
    cos_d = nc.dram_tensor("cos64", [64, T], bf16, kind="ExternalInput")
    sin_d = nc.dram_tensor("sin64", [64, T], bf16, kind="ExternalInput")
    mask_d = nc.dram_tensor("maskr", [128, 2, 128], bf16, kind="ExternalInput")
    y_d = nc.dram_tensor("y", [T, C], bf16, kind="ExternalOutput")

    xT_t = xT_d.rearrange("(k p) t -> p k t", p=128)      # [128, 8, T]
    wqkv_t = wqkv_d.rearrange("(k p) n -> p k n", p=128)  # [128, 8, 896]
    wout_t = wout_d.rearrange("(k p) n -> p k n", p=128)  # [128, 2, C]

    with tile.TileContext(nc) as tc:
        from contextlib import ExitStack

        est = ExitStack()
        with est:
            singles = est.enter_context(tc.tile_pool(name="singles", bufs=1))

            # ---- persistent SBUF tensors ----
            xT = singles.tile([128, 8, T], bf16, tag="xT")
            wqkv = singles.tile([128, 8, 896], bf16, tag="wqkv")
            wout = singles.tile([128, 2, C], bf16, tag="wout")
            cos64 = singles.tile([64, T], bf16, tag="cos64")
            sin64 = singles.tile([64, T], bf16, tag="sin64")
            maskr = singles.tile([128, 2, 128], bf16, tag="maskr")
            wu_sb = singles.tile([128, TB], bf16, tag="wu_sb")
            rot = singles.tile([128, T], bf16, tag="rot")
            qk = [singles.tile([128, T], bf16, tag=f"qk{i}", name=f"qk{i}")
                  for i in range(4)]
            qA, qB, kA, kB = qk
            # v + ones column, per head: [t, 4 heads x 65]
            vsb = singles.tile([128, NKT, HPC, 65], bf16, tag="vsb")
            m14 = [singles.tile([64, T], bf16, tag=f"m{i}", name=f"m{i}")
                   for i in range(4)]
            rotE = singles.tile([64, T], bf16, tag="rotE")
            rotO = singles.tile([64, T], bf16, tag="rotO")
            rot2 = singles.tile([64, T], bf16, tag="rot2")
            # normalized attention outputs O^T: [dh, tq] (2 heads per tile)
            oT = [[singles.tile([128, TB], bf16, tag=f"oT{p}_{b}", name=f"oT{p}_{b}")
                   for b in range(NTB)] for p in range(2)]

            for kk in range(8):
                nc.sync.dma_start(xT[:, kk], xT_t[:, kk])
                nc.sync.dma_start(wqkv[:, kk], wqkv_t[:, kk])
            nc.sync.dma_start(wout[:], wout_t[:])
            nc.sync.dma_start(cos64[:], cos_d[:])
            nc.sync.dma_start(sin64[:], sin_d[:])
            nc.sync.dma_start(maskr[:], mask_d[:])

            # ones columns of v (col 64 of each head's 65-block)
            nc.vector.memset(vsb[:, :, :, 64], 1.0)

            # ---- PE warm-up burst (HAM un-throttle while DMAs load) ----
            nc.vector.memset(wu_sb[:], 0.0)
            with tc.tile_pool(name="ps_wu", bufs=1, space="PSUM") as ps_wu:
                wu_ps = ps_wu.tile([128, TB], fp32, tag="wu")
                for _ in range(24):
                    nc.tensor.matmul(wu_ps[:], wu_sb[:, 0:128], wu_sb[:],
                                     start=True, stop=True)

            # ---- QKV projection ----
            # q,k in [d, t]: lhsT = W columns (stationary), rhs = xT (moving)
            # psum per (M-tile, t-block); M0 = rotary [q1|k1|q2|k2]
            with tc.tile_pool(name="ps_qkv", bufs=4, space="PSUM") as ps_qkv, \
                 tc.tile_pool(name="ps_v", bufs=2, space="PSUM") as ps_v:
                for mt in range(5):
                    pt = [ps_qkv.tile([128, TB], fp32, tag="ps_qkv", name=f"pqkv{mt}_{t}")
                          for t in range(NTB)]
                    for kk in range(8):
                        for tb in range(NTB):
                            nc.tensor.matmul(
                                pt[tb][:],
                                wqkv[:, kk, mt * 128:(mt + 1) * 128],
                                xT[:, kk, tb * TB:(tb + 1) * TB],
                                start=(kk == 0), stop=(kk == 7),
                            )
                    for tb in range(NTB):
                        sl = slice(tb * TB, (tb + 1) * TB)
                        if mt == 0:
                            nc.scalar.copy(rot[:, sl], pt[tb][:])
                        else:
                            # pass dims at rows 0:48 / 64:112 of the M-tile
                            # (32-aligned starts; rows 48:64 / 112:128 of the
                            # destination are rope outputs, written later)
                            nc.scalar.copy(
                                qk[mt - 1][0:48, sl], pt[tb][0:48])
                            nc.scalar.copy(
                                qk[mt - 1][64:112, sl], pt[tb][64:112])

                # v in [t, d]: lhsT = xT chunk (stationary), rhs = W v-cols
                for tch in range(NKT):
                    vp = ps_v.tile([128, 256], fp32, tag="ps_v")
                    for kk in range(8):
                        nc.tensor.matmul(
                            vp[:],
                            xT[:, kk, tch * 128:(tch + 1) * 128],
                            wqkv[:, kk, 640:896],
                            start=(kk == 0), stop=(kk == 7),
                        )
                    # strided copy into per-head 65-blocks
                    nc.scalar.copy(
                        vsb[:, tch, :, 0:64],
                        vp.rearrange("p (h d) -> p h d", h=HPC),
                    )

            # ---- RoPE ----
            # X1 = rot[0:64] (q1|k1), X2 = rot[64:128] (q2|k2).
            # All DVE ops are base-partition-0 aligned; partition moves go
            # through SBUF->SBUF DMA (arbitrary partition shift).
            nc.sync.dma_start(rot2[:], rot[64:128, :])
            nc.vector.tensor_mul(m14[0][:], rot[0:64, :], cos64[:])
            nc.vector.tensor_mul(m14[1][:], rot2[:], sin64[:])
            nc.vector.tensor_mul(m14[2][:], rot[0:64, :], sin64[:])
            nc.vector.tensor_mul(m14[3][:], rot2[:], cos64[:])
            nc.vector.tensor_sub(rotE[:], m14[0][:], m14[1][:])
            nc.vector.tensor_add(rotO[:], m14[2][:], m14[3][:])
            for j in range(8):
                i = j % 4
                dst = (qA if i < 2 else qB) if j < 4 else (kA if i < 2 else kB)
                band = 64 * (i % 2)
                off = 8 * i if j < 4 else 32 + 8 * i
                nc.sync.dma_start(
                    dst[band + 48:band + 56, :], rotE[off:off + 8, :])
                nc.sync.dma_start(
                    dst[band + 56:band + 64, :], rotO[off:off + 8, :])

            # ---- attention + output projection ----
            # A head pair (h, h+1) shares one [128, 1024] S^T psum per tk
            # tile: h0 in cols 0:512, h1 in cols 512:1024. One exp covers
            # both heads; the two S^T matmuls (K=64, partition bands 0/64)
            # are emitted adjacently so they run concurrently in separate
            # PE row-quadrants. Diagonal tiles compute only live causal
            # columns; partial triangle squares get one strided mask mul.
            with tc.tile_pool(name="ps_s", bufs=2, space="PSUM") as ps_s, \
                 tc.tile_pool(name="ps_o", bufs=2, space="PSUM") as ps_o, \
                 tc.tile_pool(name="ps_y", bufs=2, space="PSUM") as ps_y, \
                 tc.tile_pool(name="p_pool", bufs=4) as p_pool, \
                 tc.tile_pool(name="r_pool", bufs=2) as r_pool:
                scale = 1.0 / float(np.sqrt(D_HEAD))

                def s_tile(hls, b, j):
                    """S^T for tk tile j, both heads of the pair, one psum."""
                    lo = 128 * (j - 4 * b) if j >= 4 * b else 0
                    qsl0 = b * TB
                    sp = ps_s.tile([128, 2 * TB], fp32, tag="ps_s",
                                   name=f"sp{hls[0]}_{b}_{j}")
                    for col, hl in enumerate(hls):
                        qt = qk[0] if hl < 2 else qk[1]
                        kt = qk[2] if hl < 2 else qk[3]
                        band = 64 * (hl % 2)
                        nc.tensor.matmul(
                            sp[:, col * TB + lo:(col + 1) * TB],
                            kt[band:band + 64, j * 128:(j + 1) * 128],
                            qt[band:band + 64, qsl0 + lo:qsl0 + TB],
                            start=True, stop=True,
                        )
                    return sp

                def norm_unit(b, hls, ops):
                    """Normalize O^T for both heads of a unit (reads psum)."""
                    for hl in hls:
                        op = ops[hl]
                        band = 64 * (hl % 2)
                        rc = r_pool.tile([1, TB], fp32, tag="rc",
                                         name=f"rc{hl}_{b}", bufs=2)
                        rb = r_pool.tile([64, TB], fp32, tag="rb",
                                         name=f"rb{hl}_{b}", bufs=2)
                        nc.vector.tensor_copy(rc[:], op[64:65, :])
                        nc.gpsimd.partition_broadcast(rb[:], rc[:])
                        nc.vector.reciprocal_approx_fast(rb[:], rb[:])
                        if band == 0:
                            nc.vector.tensor_mul(
                                oT[hl // 2][b][0:64, :], op[0:64, :], rb[:])
                        else:
                            nrm = r_pool.tile([64, TB], bf16, tag="nrm",
                                              name=f"nrm{hl}_{b}", bufs=2)
                            nc.vector.tensor_mul(
                                nrm[:], op[0:64, :], rb[:])
                            nc.sync.dma_start(
                                oT[hl // 2][b][64:128, :], nrm[:])

                def outproj_chunk(b, tc_):
                    """Output projection for one 128-row tq chunk."""
                    csl = slice((tc_ % 4) * 128, (tc_ % 4) * 128 + 128)
                    yp0 = ps_y.tile([128, TB], fp32, tag="yp",
                                    name=f"yp0_{tc_}")
                    yp1 = ps_y.tile([128, TB], fp32, tag="yp",
                                    name=f"yp1_{tc_}")
                    for i in range(2):
                        nc.tensor.matmul(
                            yp0[:], oT[i][b][:, csl], wout[:, i, 0:512],
                            start=(i == 0), stop=(i == 1))
                        nc.tensor.matmul(
                            yp1[:], oT[i][b][:, csl], wout[:, i, 512:1024],
                            start=(i == 0), stop=(i == 1))
                    ysb = r_pool.tile([128, C], bf16, tag="ysb",
                                      name=f"ysb{tc_}", bufs=2)
                    nc.vector.tensor_copy(ysb[:, 0:512], yp0[:])
                    nc.vector.tensor_copy(ysb[:, 512:1024], yp1[:])
                    nc.sync.dma_start(
                        y_d[tc_ * 128:(tc_ + 1) * 128, :], ysb[:])

                fillers = []
                for b in range(NTB):
                    njt = 4 * b + 4
                    for pg in (0, 2):
                        hls = (pg, pg + 1)
                        ops = {hl: ps_o.tile([65, TB], fp32, tag="ps_o",
                                             name=f"op{hl}_{b}")
                               for hl in hls}
                        sp = s_tile(hls, b, 0)
                        for j in range(njt):
                            lo = 128 * (j - 4 * b) if j >= 4 * b else 0
                            pb = p_pool.tile([128, 2, TB], bf16, tag="pb",
                                             name=f"pb{pg}_{b}_{j}")
                            if lo:
                                nc.scalar.activation(
                                    pb[:, :, lo:TB], sp.rearrange(
                                        "p (h n) -> p h n", h=2)[:, :, lo:TB],
                                    mybir.ActivationFunctionType.Exp,
                                    scale=scale)
                            else:
                                nc.scalar.activation(
                                    pb.rearrange("p h n -> p (h n)"), sp[:],
                                    mybir.ActivationFunctionType.Exp,
                                    scale=scale)
                            if j >= 4 * b:
                                r = j - 4 * b
                                nc.gpsimd.tensor_mul(
                                    pb[:, :, 128 * r:128 * r + 128],
                                    pb[:, :, 128 * r:128 * r + 128],
                                    maskr[:])
                            spn = s_tile(hls, b, j + 1) if j + 1 < njt else None
                            for col, hl in enumerate(hls):
                                nc.tensor.matmul(
                                    ops[hl][:, lo:TB],
                                    vsb[:, j, hl, :],
                                    pb[:, col, lo:TB],
                                    start=(j == 0), stop=(j == njt - 1),
                                    skip_group_check=(lo > 0),
                                )
                            if spn is not None:
                                sp = spn
                            if fillers:
                                bb, cc = fillers.pop(0)
                                outproj_chunk(bb, cc)
                        # normalize inline (frees the op psum slots);
                        # outproj chunks are deferred as fillers, popped one
                        # per j-step inside the next units' inner loops
                        norm_unit(b, hls, ops)
                        if hls[0] == 2:
                            fillers.extend((b, tc_)
                                           for tc_ in range(4 * b, 4 * b + 4))
                for bb, cc in fillers:
                    outproj_chunk(bb, cc)

    nc.compile()
    _nc_cache["nc"] = nc
    return nc


def _host_inputs(x, w_qkv, w_out, rope_sin, rope_cos):
    """Per-core input maps (host-side shard + layout prep)."""
    bf = _BF16
    xb = np.asarray(x, dtype=np.float32).astype(bf)  # [B, T, C]
    wq = np.asarray(w_qkv, dtype=np.float32).astype(bf)  # [C, 3C]
    wo = np.asarray(w_out, dtype=np.float32).astype(bf)  # [C, C]
    sin = np.asarray(rope_sin, dtype=np.float32).astype(bf)  # [T, 8]
    cos = np.asarray(rope_cos, dtype=np.float32).astype(bf)

    cos64 = np.ascontiguousarray(np.tile(cos.T, (8, 1)))  # [64, T]
    sin64 = np.ascontiguousarray(np.tile(sin.T, (8, 1)))

    # binary lower-triangle square mask [tk, tq], duplicated per head pair
    tk = np.arange(128)[:, None]
    tq = np.arange(128)[None, :]
    msq = (tq >= tk).astype(bf)
    maskr = np.stack([msq, msq], axis=1)  # [128, 2, 128]

    evens = list(range(0, ROT_DIM, 2))
    odds = list(range(1, ROT_DIM, 2))
    rest = list(range(ROT_DIM, D_HEAD))
    perm = evens + odds + rest  # within-head d permutation for q, k

    in_maps = []
    for c in range(N_CORES):
        bi, hg = c // HPC, c % HPC
        heads = [HPC * hg + i for i in range(HPC)]
        cols = []
        # M0: q1 | k1 | q2 | k2 (4 heads x 8 each)
        for base, sub in ((0, evens), (C, evens), (0, odds), (C, odds)):
            for h in heads:
                cols += [base + h * D_HEAD + d for d in sub]
        wqkv_c = np.zeros((C, 896), dtype=bf)
        wqkv_c[:, 0:128] = wq[:, cols]
        # pass M-tiles 1-4: two 48-row head blocks at rows 16:64 and 80:128
        for blk in range(8):  # q h0..h3, k h0..h3
            base = 0 if blk < 4 else C
            h = heads[blk % 4]
            mt = 1 + blk // 2
            r0 = 0 if blk % 2 == 0 else 64
            wqkv_c[:, 128 * mt + r0:128 * mt + r0 + 48] = \
                wq[:, [base + h * D_HEAD + d for d in rest]]
        # v, plain order, cols 640:896
        vcols = [2 * C + h * D_HEAD + d for h in heads for d in range(D_HEAD)]
        wqkv_c[:, 640:896] = wq[:, vcols]
        wout_c = np.ascontiguousarray(
            wo[[h * D_HEAD + d for h in heads for d in range(D_HEAD)], :])
        xT = np.ascontiguousarray(xb[bi].T)  # [C, T]
        in_maps.append({
            "xT": xT, "wqkv": wqkv_c, "wout": wout_c,
            "cos64": cos64, "sin64": sin64, "maskr": maskr,
        })
    return in_maps


def kernel(x, mask, w_qkv, w_out, rope_sin, rope_cos):
    global LAST_EXEC_NS
    mask = np.asarray(mask)
    causal = np.array_equal(
        mask.reshape(T, T), np.tril(np.ones((T, T), dtype=bool)))
    if not causal or _BF16 is None:
        return _kernel_numpy(x, mask, w_qkv, w_out, rope_sin, rope_cos)

    _install_ntff_hook()
    nc = _build_bass()
    from concourse.bass_utils import run_bass_kernel_spmd

    in_maps = _host_inputs(x, w_qkv, w_out, rope_sin, rope_cos)
    res = run_bass_kernel_spmd(
        nc, in_maps, core_ids=list(range(N_CORES)),
        trace=bool(os.environ.get("BASS_TRACE")),
    )
    LAST_EXEC_NS = res.exec_time_ns

    y = np.empty((B, T, C), dtype=np.float32)
    for bi in range(B):
        acc = np.zeros((T, C), dtype=np.float32)
        for hg in range(HPC):
            acc += np.asarray(res.results[bi * HPC + hg]["y"],
                              dtype=np.float32)
        y[bi] = acc
    return y


# revision 19
# speedup vs baseline: 1.2010x; 1.2010x over previous
"""nn_Attention kernel — dense transformer attention block on 8 TRN2 NeuronCores.

Contract: kernel(**inputs) takes FULL unsharded inputs and returns the FULL
output. Sharding: tensor-parallel over heads x data-parallel over batch.
Core c = bi*4 + hg handles batch bi, heads 4*hg..4*hg+3. Each core computes
its QKV column slice, RoPE, causal attention, and a row-slice partial of the
output projection; partials are summed on the host (the all-reduce) and
batches concatenated.

Kernel layout notes:
 - q,k are produced directly in [d, t] layout (W stationary, xT moving) so
   the S^T = K Q^T matmul needs no transposes. v is produced in [t, d]
   (xT stationary) to serve as the PV lhsT.
 - RoPE: host permutes W_q/W_k columns per head to [evens, odds, pass] and
   the first QKV M-tile is [q1 | k1 | q2 | k2] (4 heads x 8 rows each), so
   rotation is 4 batched DVE muls + 16 small writes into the final tiles.
   The d-permutation cancels in q.k dot products.
 - Softmax is computed unnormalized (exp, no max subtraction — scores are
   O(1) so exp cannot overflow); a ones column appended to V makes the
   denominator row 64 of the PV output, and normalization is applied after
   PV via reciprocal + partition_broadcast.
 - Causality: tk tiles strictly above the tq block are skipped outright;
   the 4 diagonal tiles per block are fixed with a binary mask multiply.
"""

import os
import sys
import types

import numpy as np

try:
    import ml_dtypes

    _BF16 = ml_dtypes.bfloat16
except Exception:  # pragma: no cover
    _BF16 = None

D_HEAD = 64
N_HEADS = 16
ROT_DIM = 16  # D_HEAD * 0.25
B, T, C = 2, 2048, N_HEADS * D_HEAD
HPC = 4  # heads per core
N_CORES = 8
TB = 512  # tq block size
NTB = T // TB  # 4 tq blocks
NKT = T // 128  # 16 tk tiles

_nc_cache = {}
LAST_EXEC_NS = None


# ----------------------------------------------------------------------------
# numpy reference fallback (used only if the mask is not causal-tril)
# ----------------------------------------------------------------------------

def _bf(a):
    if _BF16 is None:
        return np.float32(a)
    return np.asarray(a).astype(_BF16).astype(np.float32)


def _softmax_f32(s):
    s = s - s.max(axis=-1, keepdims=True)
    e = np.exp(s, dtype=np.float32)
    return e / e.sum(axis=-1, keepdims=True)


def _kernel_numpy(x, mask, w_qkv, w_out, rope_sin, rope_cos):
    x = np.asarray(x, dtype=np.float32)
    mask = np.asarray(mask)
    sin = _bf(np.asarray(rope_sin, dtype=np.float32))
    cos = _bf(np.asarray(rope_cos, dtype=np.float32))
    b, t, c = x.shape
    add_mask = (1.0 - mask.astype(np.float32)) * -1e10
    xb = _bf(x)
    wq = _bf(np.asarray(w_qkv, dtype=np.float32))
    wo = _bf(np.asarray(w_out, dtype=np.float32))
    qkv = _bf(np.einsum("btc,cd->btd", xb, wq, optimize=True))
    qkv = qkv.reshape(b, t, 3, N_HEADS, D_HEAD)
    q, k, v = qkv[:, :, 0], qkv[:, :, 1], qkv[:, :, 2]
    sin4 = sin[None, :, None, :]
    cos4 = cos[None, :, None, :]

    def rope(z):
        z_rot, z_pass = z[..., :ROT_DIM], z[..., ROT_DIM:]
        z1, z2 = z_rot[..., 0::2], z_rot[..., 1::2]
        zr = np.concatenate(
            [_bf(_bf(z1 * cos4) - _bf(z2 * sin4)),
             _bf(_bf(z1 * sin4) + _bf(z2 * cos4))], axis=-1
        )
        return np.concatenate([zr, z_pass], axis=-1)

    q = rope(q)
    k = rope(k)
    out_heads = np.empty((b, t, N_HEADS, D_HEAD), dtype=np.float32)
    scale = 1.0 / np.sqrt(np.float32(D_HEAD))
    for bi in range(b):
        for h in range(N_HEADS):
            s = _bf(q[bi, :, h] @ k[bi, :, h].T)
            s = _bf(s * scale)
            s = s.astype(np.float32) + add_mask[0, 0]
            p = _bf(_softmax_f32(s))
            out_heads[bi, :, h] = _bf(p @ v[bi, :, h])
    out = out_heads.reshape(b * t, c)
    y = _bf(out @ wo)
    return y.reshape(b, t, c).astype(np.float32)


# ----------------------------------------------------------------------------
# bass kernel
# ----------------------------------------------------------------------------

def _install_ntff_hook():
    """Shim antenv.axon_hooks (missing in this image) so trace=True works."""
    if "antenv.axon_hooks" in sys.modules:
        return
    hook = [None]
    mod = types.ModuleType("antenv.axon_hooks")
    mod.set_axon_ntff_profile_hook = lambda h: hook.__setitem__(0, h)
    mod.get_axon_ntff_profile_hook = lambda: hook[0]
    sys.modules["antenv.axon_hooks"] = mod
    try:
        from trn_agent_boot.trn_boot import _ntff_profile_via_ctypes

        mod.set_axon_ntff_profile_hook(
            _ntff_profile_via_ctypes("/opt/axon/libaxon_pjrt.so")
        )
    except Exception:
        pass


def _build_bass():
    """Build + compile the single-core SPMD program (same for all 8 cores)."""
    if "nc" in _nc_cache:
        return _nc_cache["nc"]

    import concourse.bass as bass
    import concourse.mybir as mybir
    import concourse.tile as tile
    from concourse import bacc

    fp32 = mybir.dt.float32
    bf16 = mybir.dt.bfloat16

    nc = bacc.Bacc("TRN2", target_bir_lowering=False, debug=False,
                   num_devices=N_CORES)

    xT_d = nc.dram_tensor("xT", [C, T], bf16, kind="ExternalInput")
    wqkv_d = nc.dram_tensor("wqkv", [C, 896], bf16, kind="ExternalInput")
    wout_d = nc.dram_tensor("wout", [HPC * D_HEAD, C], bf16, kind="ExternalInput")
    cos_d = nc.dram_tensor("cos64", [64, T], bf16, kind="ExternalInput")
    sin_d = nc.dram_tensor("sin64", [64, T], bf16, kind="ExternalInput")
    mask_d = nc.dram_tensor("maskr", [128, 2, 128], bf16, kind="ExternalInput")
    y_d = nc.dram_tensor("y", [T, C], bf16, kind="ExternalOutput")

    xT_t = xT_d.rearrange("(k p) t -> p k t", p=128)      # [128, 8, T]
    wqkv_t = wqkv_d.rearrange("(k p) n -> p k n", p=128)  # [128, 8, 896]
    wout_t = wout_d.rearrange("(k p) n -> p k n", p=128)  # [128, 2, C]

    with tile.TileContext(nc) as tc:
        from contextlib import ExitStack

        est = ExitStack()
        with est:
            singles = est.enter_context(tc.tile_pool(name="singles", bufs=1))

            # ---- persistent SBUF tensors ----
            xT = singles.tile([128, 8, T], bf16, tag="xT")
            wqkv = singles.tile([128, 8, 896], bf16, tag="wqkv")
            wout = singles.tile([128, 2, C], bf16, tag="wout")
            cos64 = singles.tile([64, T], bf16, tag="cos64")
            sin64 = singles.tile([64, T], bf16, tag="sin64")
            maskr = singles.tile([128, 2, 128], bf16, tag="maskr")
            wu_sb = singles.tile([128, TB], bf16, tag="wu_sb")
            rot = singles.tile([128, T], bf16, tag="rot")
            qk = [singles.tile([128, T], bf16, tag=f"qk{i}", name=f"qk{i}")
                  for i in range(4)]
            qA, qB, kA, kB = qk
            # v + ones column, per head: [t, 4 heads x 65]
            vsb = singles.tile([128, NKT, HPC, 65], bf16, tag="vsb")
            m14 = [singles.tile([64, T], bf16, tag=f"m{i}", name=f"m{i}")
                   for i in range(4)]
            rotE = singles.tile([64, T], bf16, tag="rotE")
            rotO = singles.tile([64, T], bf16, tag="rotO")
            rot2 = singles.tile([64, T], bf16, tag="rot2")
            # normalized attention outputs O^T: [dh, tq] (2 heads per tile)
            oT = [[singles.tile([128, TB], bf16, tag=f"oT{p}_{b}", name=f"oT{p}_{b}")
                   for b in range(NTB)] for p in range(2)]

            for kk in range(8):
                nc.sync.dma_start(xT[:, kk], xT_t[:, kk])
                nc.sync.dma_start(wqkv[:, kk], wqkv_t[:, kk])
            nc.sync.dma_start(wout[:], wout_t[:])
            nc.sync.dma_start(cos64[:], cos_d[:])
            nc.sync.dma_start(sin64[:], sin_d[:])
            nc.sync.dma_start(maskr[:], mask_d[:])

            # ones columns of v (col 64 of each head's 65-block)
            nc.vector.memset(vsb[:, :, :, 64], 1.0)

            # ---- PE warm-up burst (HAM un-throttle while DMAs load) ----
            nc.vector.memset(wu_sb[:], 0.0)
            with tc.tile_pool(name="ps_wu", bufs=1, space="PSUM") as ps_wu:
                wu_ps = ps_wu.tile([128, TB], fp32, tag="wu")
                for _ in range(24):
                    nc.tensor.matmul(wu_ps[:], wu_sb[:, 0:128], wu_sb[:],
                                     start=True, stop=True)

            # ---- QKV projection ----
            # q,k in [d, t]: lhsT = W columns (stationary), rhs = xT (moving)
            # psum per (M-tile, t-block); M0 = rotary [q1|k1|q2|k2]
            with tc.tile_pool(name="ps_qkv", bufs=4, space="PSUM") as ps_qkv, \
                 tc.tile_pool(name="ps_v", bufs=2, space="PSUM") as ps_v:
                for mt in range(5):
                    pt = [ps_qkv.tile([128, TB], fp32, tag="ps_qkv", name=f"pqkv{mt}_{t}")
                          for t in range(NTB)]
                    for kk in range(8):
                        for tb in range(NTB):
                            nc.tensor.matmul(
                                pt[tb][:],
                                wqkv[:, kk, mt * 128:(mt + 1) * 128],
                                xT[:, kk, tb * TB:(tb + 1) * TB],
                                start=(kk == 0), stop=(kk == 7),
                            )
                    for tb in range(NTB):
                        sl = slice(tb * TB, (tb + 1) * TB)
                        if mt == 0:
                            nc.scalar.copy(rot[:, sl], pt[tb][:])
                        else:
                            # pass dims at rows 0:48 / 64:112 of the M-tile
                            # (32-aligned starts; rows 48:64 / 112:128 of the
                            # destination are rope outputs, written later)
                            nc.scalar.copy(
                                qk[mt - 1][0:48, sl], pt[tb][0:48])
                            nc.scalar.copy(
                                qk[mt - 1][64:112, sl], pt[tb][64:112])

                # v in [t, d]: lhsT = xT chunk (stationary), rhs = W v-cols
                for tch in range(NKT):
                    vp = ps_v.tile([128, 256], fp32, tag="ps_v")
                    for kk in range(8):
                        nc.tensor.matmul(
                            vp[:],
                            xT[:, kk, tch * 128:(tch + 1) * 128],
                            wqkv[:, kk, 640:896],
                            start=(kk == 0), stop=(kk == 7),
                        )
                    # strided copy into per-head 65-blocks
                    nc.scalar.copy(
                        vsb[:, tch, :, 0:64],
                        vp.rearrange("p (h d) -> p h d", h=HPC),
                    )

            # ---- RoPE ----
            # X1 = rot[0:64] (q1|k1), X2 = rot[64:128] (q2|k2).
            # All DVE ops are base-partition-0 aligned; partition moves go
            # through SBUF->SBUF DMA (arbitrary partition shift).
            nc.sync.dma_start(rot2[:], rot[64:128, :])
            nc.vector.tensor_mul(m14[0][:], rot[0:64, :], cos64[:])
            nc.vector.tensor_mul(m14[1][:], rot2[:], sin64[:])
            nc.vector.tensor_mul(m14[2][:], rot[0:64, :], sin64[:])
            nc.vector.tensor_mul(m14[3][:], rot2[:], cos64[:])
            nc.vector.tensor_sub(rotE[:], m14[0][:], m14[1][:])
            nc.vector.tensor_add(rotO[:], m14[2][:], m14[3][:])
            for j in range(8):
                i = j % 4
                dst = (qA if i < 2 else qB) if j < 4 else (kA if i < 2 else kB)
                band = 64 * (i % 2)
                off = 8 * i if j < 4 else 32 + 8 * i
                nc.sync.dma_start(
                    dst[band + 48:band + 56, :], rotE[off:off + 8, :])
                nc.sync.dma_start(
                    dst[band + 56:band + 64, :], rotO[off:off + 8, :])

            # ---- attention + output projection ----
            # A head pair (h, h+1) shares one [128, 1024] S^T psum per tk
            # tile: h0 in cols 0:512, h1 in cols 512:1024. One exp covers
            # both heads; the two S^T matmuls (K=64, partition bands 0/64)
            # are emitted adjacently so they run concurrently in separate
            # PE row-quadrants. Diagonal tiles compute only live causal
            # columns; partial triangle squares get one strided mask mul.
            with tc.tile_pool(name="ps_s", bufs=2, space="PSUM") as ps_s, \
                 tc.tile_pool(name="ps_o", bufs=2, space="PSUM") as ps_o, \
                 tc.tile_pool(name="ps_y", bufs=2, space="PSUM") as ps_y, \
                 tc.tile_pool(name="p_pool", bufs=4) as p_pool, \
                 tc.tile_pool(name="r_pool", bufs=2) as r_pool:
                scale = 1.0 / float(np.sqrt(D_HEAD))

                def s_tile(hls, b, j):
                    """S^T for tk tile j, both heads of the pair, one psum."""
                    lo = 128 * (j - 4 * b) if j >= 4 * b else 0
                    qsl0 = b * TB
                    sp = ps_s.tile([128, 2 * TB], fp32, tag="ps_s",
                                   name=f"sp{hls[0]}_{b}_{j}")
                    for col, hl in enumerate(hls):
                        qt = qk[0] if hl < 2 else qk[1]
                        kt = qk[2] if hl < 2 else qk[3]
                        band = 64 * (hl % 2)
                        nc.tensor.matmul(
                            sp[:, col * TB + lo:(col + 1) * TB],
                            kt[band:band + 64, j * 128:(j + 1) * 128],
                            qt[band:band + 64, qsl0 + lo:qsl0 + TB],
                            start=True, stop=True,
                        )
                    return sp

                def norm_unit(b, hls, ops):
                    """Normalize O^T for both heads of a unit (reads psum)."""
                    for hl in hls:
                        op = ops[hl]
                        band = 64 * (hl % 2)
                        rc = r_pool.tile([1, TB], fp32, tag="rc",
                                         name=f"rc{hl}_{b}", bufs=2)
                        rb = r_pool.tile([64, TB], fp32, tag="rb",
                                         name=f"rb{hl}_{b}", bufs=2)
                        nc.vector.tensor_copy(rc[:], op[64:65, :])
                        nc.gpsimd.partition_broadcast(rb[:], rc[:])
                        nc.vector.reciprocal_approx_fast(rb[:], rb[:])
                        if band == 0:
                            nc.vector.tensor_mul(
                                oT[hl // 2][b][0:64, :], op[0:64, :], rb[:])
                        else:
                            nrm = r_pool.tile([64, TB], bf16, tag="nrm",
                                              name=f"nrm{hl}_{b}", bufs=2)
                            nc.vector.tensor_mul(
                                nrm[:], op[0:64, :], rb[:])
                            nc.sync.dma_start(
                                oT[hl // 2][b][64:128, :], nrm[:])

                def outproj_chunk(b, tc_):
                    """Output projection for one 128-row tq chunk."""
                    csl = slice((tc_ % 4) * 128, (tc_ % 4) * 128 + 128)
                    yp0 = ps_y.tile([128, TB], fp32, tag="yp",
                                    name=f"yp0_{tc_}")
                    yp1 = ps_y.tile([128, TB], fp32, tag="yp",
                                    name=f"yp1_{tc_}")
                    for i in range(2):
                        nc.tensor.matmul(
                            yp0[:], oT[i][b][:, csl], wout[:, i, 0:512],
                            start=(i == 0), stop=(i == 1))
                        nc.tensor.matmul(
                            yp1[:], oT[i][b][:, csl], wout[:, i, 512:1024],
                            start=(i == 0), stop=(i == 1))
                    ysb = r_pool.tile([128, C], bf16, tag="ysb",
                                      name=f"ysb{tc_}", bufs=2)
                    nc.vector.tensor_copy(ysb[:, 0:512], yp0[:])
                    nc.vector.tensor_copy(ysb[:, 512:1024], yp1[:])
                    nc.sync.dma_start(
                        y_d[tc_ * 128:(tc_ + 1) * 128, :], ysb[:])

                fillers = []
                for b in range(NTB):
                    njt = 4 * b + 4
                    for pg in (0, 2):
                        hls = (pg, pg + 1)
                        ops = {hl: ps_o.tile([65, TB], fp32, tag="ps_o",
                                             name=f"op{hl}_{b}")
                               for hl in hls}
                        sp = s_tile(hls, b, 0)
                        for j in range(njt):
                            lo = 128 * (j - 4 * b) if j >= 4 * b else 0
                            pb = p_pool.tile([128, 2, TB], bf16, tag="pb",
                                             name=f"pb{pg}_{b}_{j}")
                            if lo:
                                nc.scalar.activation(
                                    pb[:, :, lo:TB], sp.rearrange(
                                        "p (h n) -> p h n", h=2)[:, :, lo:TB],
                                    mybir.ActivationFunctionType.Exp,
                                    scale=scale)
                            else:
                                nc.scalar.activation(
                                    pb.rearrange("p h n -> p (h n)"), sp[:],
                                    mybir.ActivationFunctionType.Exp,
                                    scale=scale)
                            if j >= 4 * b:
                                r = j - 4 * b
                                nc.vector.tensor_mul(
                                    pb[:, :, 128 * r:128 * r + 128],
                                    pb[:, :, 128 * r:128 * r + 128],
                                    maskr[:])
                            spn = s_tile(hls, b, j + 1) if j + 1 < njt else None
                            for col, hl in enumerate(hls):
                                nc.tensor.matmul(
                                    ops[hl][:, lo:TB],
                                    vsb[:, j, hl, :],
                                    pb[:, col, lo:TB],
                                    start=(j == 0), stop=(j == njt - 1),
                                    skip_group_check=(lo > 0),
                                )
                            if spn is not None:
                                sp = spn
                            if fillers:
                                bb, cc = fillers.pop(0)
                                outproj_chunk(bb, cc)
                        # normalize inline (frees the op psum slots);
                        # outproj chunks are deferred as fillers, popped one
                        # per j-step inside the next units' inner loops
                        norm_unit(b, hls, ops)
                        if hls[0] == 2:
                            fillers.extend((b, tc_)
                                           for tc_ in range(4 * b, 4 * b + 4))
                for bb, cc in fillers:
                    outproj_chunk(bb, cc)

    nc.compile()
    _nc_cache["nc"] = nc
    return nc


def _host_inputs(x, w_qkv, w_out, rope_sin, rope_cos):
    """Per-core input maps (host-side shard + layout prep)."""
    bf = _BF16
    xb = np.asarray(x, dtype=np.float32).astype(bf)  # [B, T, C]
    wq = np.asarray(w_qkv, dtype=np.float32).astype(bf)  # [C, 3C]
    wo = np.asarray(w_out, dtype=np.float32).astype(bf)  # [C, C]
    sin = np.asarray(rope_sin, dtype=np.float32).astype(bf)  # [T, 8]
    cos = np.asarray(rope_cos, dtype=np.float32).astype(bf)

    cos64 = np.ascontiguousarray(np.tile(cos.T, (8, 1)))  # [64, T]
    sin64 = np.ascontiguousarray(np.tile(sin.T, (8, 1)))

    # binary lower-triangle square mask [tk, tq], duplicated per head pair
    tk = np.arange(128)[:, None]
    tq = np.arange(128)[None, :]
    msq = (tq >= tk).astype(bf)
    maskr = np.stack([msq, msq], axis=1)  # [128, 2, 128]

    evens = list(range(0, ROT_DIM, 2))
    odds = list(range(1, ROT_DIM, 2))
    rest = list(range(ROT_DIM, D_HEAD))
    perm = evens + odds + rest  # within-head d permutation for q, k

    in_maps = []
    for c in range(N_CORES):
        bi, hg = c // HPC, c % HPC
        heads = [HPC * hg + i for i in range(HPC)]
        cols = []
        # M0: q1 | k1 | q2 | k2 (4 heads x 8 each)
        for base, sub in ((0, evens), (C, evens), (0, odds), (C, odds)):
            for h in heads:
                cols += [base + h * D_HEAD + d for d in sub]
        wqkv_c = np.zeros((C, 896), dtype=bf)
        wqkv_c[:, 0:128] = wq[:, cols]
        # pass M-tiles 1-4: two 48-row head blocks at rows 16:64 and 80:128
        for blk in range(8):  # q h0..h3, k h0..h3
            base = 0 if blk < 4 else C
            h = heads[blk % 4]
            mt = 1 + blk // 2
            r0 = 0 if blk % 2 == 0 else 64
            wqkv_c[:, 128 * mt + r0:128 * mt + r0 + 48] = \
                wq[:, [base + h * D_HEAD + d for d in rest]]
        # v, plain order, cols 640:896
        vcols = [2 * C + h * D_HEAD + d for h in heads for d in range(D_HEAD)]
        wqkv_c[:, 640:896] = wq[:, vcols]
        wout_c = np.ascontiguousarray(
            wo[[h * D_HEAD + d for h in heads for d in range(D_HEAD)], :])
        xT = np.ascontiguousarray(xb[bi].T)  # [C, T]
        in_maps.append({
            "xT": xT, "wqkv": wqkv_c, "wout": wout_c,
            "cos64": cos64, "sin64": sin64, "maskr": maskr,
        })
    return in_maps


def kernel(x, mask, w_qkv, w_out, rope_sin, rope_cos):
    global LAST_EXEC_NS
    mask = np.asarray(mask)
    causal = np.array_equal(
        mask.reshape(T, T), np.tril(np.ones((T, T), dtype=bool)))
    if not causal or _BF16 is None:
        return _kernel_numpy(x, mask, w_qkv, w_out, rope_sin, rope_cos)

    _install_ntff_hook()
    nc = _build_bass()
    from concourse.bass_utils import run_bass_kernel_spmd

    in_maps = _host_inputs(x, w_qkv, w_out, rope_sin, rope_cos)
    res = run_bass_kernel_spmd(
        nc, in_maps, core_ids=list(range(N_CORES)),
        trace=bool(os.environ.get("BASS_TRACE")),
    )
    LAST_EXEC_NS = res.exec_time_ns

    y = np.empty((B, T, C), dtype=np.float32)
    for bi in range(B):
        acc = np.zeros((T, C), dtype=np.float32)
        for hg in range(HPC):
            acc += np.asarray(res.results[bi * HPC + hg]["y"],
                              dtype=np.float32)
        y[bi] = acc
    return y
